# revision 1
# baseline (speedup 1.0000x reference)
"""CapsuleLayer kernel for Trainium2, 8 NeuronCores.

Math: the reference's softmax is over a singleton axis, so c_ij == 1 and the
routing loop is dead code.  The output is exactly

    s[b, j, k]  = sum_{i, u} W[0, i, j, k, u] * x[b, u, i]
    m[b, k]     = sum_j s[b, j, k]^2
    v[b, j, k]  = (sqrt(m) / (1 + m)) * s[b, j, k]        (squash)

i.e. one (32 x 32768) @ (32768 x 1024) fp32 matmul plus a tiny per-(b,k)
epilogue.  W (128 MiB) dominates: the kernel is HBM-bound on reading W once.

Sharding: the output column grid is (k, j) with k = unit_size (64).  Shard on
k: core c owns k in [8c, 8c+8).  Each core reads its W slice (16 MiB, read
exactly once machine-wide), the full x (4 MiB, replicated), and computes a
fully local squash (the j-reduction inside m is intact per core).  Zero
cross-core communication.

Host-side, W is resliced into the exact SBUF streaming layout
[chunk][partition=contraction%128][ktile-in-chunk x column] so every DMA is a
single large contiguous transfer.
"""

import numpy as np

B, U, I, J, K = 32, 16, 2048, 16, 64  # batch, in_units, in_ch, num_units, unit_size
NC = 8                                # cores
KPC = K // NC                         # unit_size columns per core (8)
N = KPC * J                           # output columns per core (128), kk-major, j-minor
KK = I * U                            # contraction length (32768)
P = 128                               # partitions
KT = KK // P                          # contraction tiles (256)
CH = 16                               # contraction tiles per DMA chunk (1 MiB chunks)
NCH = KT // CH                        # chunks per core (16)

_CACHE = {}


def _build():
    import concourse.bacc as bacc
    import concourse.tile as tile
    import concourse.mybir as mybir

    f32 = mybir.dt.float32
    nc = bacc.Bacc("TRN2", num_devices=NC, debug=False, enable_asserts=False)
    x_d = nc.dram_tensor("x", (P, KT * B), f32, kind="ExternalInput")
    w_d = nc.dram_tensor("w", (NCH, P, CH * N), f32, kind="ExternalInput")
    v_d = nc.dram_tensor("v", (B, KPC, J), f32, kind="ExternalOutput")

    with tile.TileContext(nc) as tc:
        with (
            tc.tile_pool(name="xp", bufs=1) as xp,
            tc.tile_pool(name="wp", bufs=3) as wp,
            tc.tile_pool(name="ep", bufs=1) as ep,
            tc.tile_pool(name="ps", bufs=1, space="PSUM") as ps,
        ):
            x_sb = xp.tile([P, KT * B], f32)
            nc.sync.dma_start(x_sb[:], x_d[:])

            s_ps = ps.tile([B, KPC, J], f32)
            for c in range(NCH):
                w_sb = wp.tile([P, CH * N], f32)
                nc.sync.dma_start(w_sb[:], w_d[c])
                for t in range(CH):
                    kt = c * CH + t
                    nc.tensor.matmul(
                        s_ps[:, :, :],
                        x_sb[:, kt * B : (kt + 1) * B],
                        w_sb[:, t * N : (t + 1) * N],
                        start=(kt == 0),
                        stop=(kt == KT - 1),
                    )

            s_sb = ep.tile([B, KPC, J], f32)
            nc.vector.tensor_copy(s_sb[:], s_ps[:])
            s2 = ep.tile([B, KPC, J], f32)
            nc.scalar.square(s2[:], s_ps[:])
            m = ep.tile([B, KPC], f32)
            nc.vector.reduce_sum(m[:], s2[:], axis=mybir.AxisListType.X)
            sq = ep.tile([B, KPC], f32)
            nc.scalar.sqrt(sq[:], m[:])
            d = ep.tile([B, KPC], f32)
            nc.vector.tensor_scalar_add(d[:], m[:], 1.0)
            r = ep.tile([B, KPC], f32)
            nc.vector.reciprocal(r[:], d[:])
            sc = ep.tile([B, KPC], f32)
            nc.vector.tensor_mul(sc[:], sq[:], r[:])
            v_sb = ep.tile([B, KPC, J], f32)
            for kk in range(KPC):
                nc.vector.tensor_scalar_mul(
                    v_sb[:, kk, :], s_sb[:, kk, :], sc[:, kk : kk + 1]
                )
            nc.sync.dma_start(v_d[:], v_sb[:])

    nc.compile()
    return nc


def get_nc():
    if "nc" not in _CACHE:
        _CACHE["nc"] = _build()
    return _CACHE["nc"]


def prep_inputs(x, W):
    """Full inputs -> per-core in_maps with the streaming layouts."""
    x = np.ascontiguousarray(np.asarray(x, dtype=np.float32))
    W = np.asarray(W, dtype=np.float32)
    assert x.shape == (B, U, I) and W.shape == (1, I, J, K, U)

    # x[b,u,i] -> [KK=(i major, u minor), b] -> [P, KT*B] (partition-major tiles)
    xm = x.transpose(2, 1, 0).reshape(KK, B)
    xh = np.ascontiguousarray(
        xm.reshape(KT, P, B).transpose(1, 0, 2).reshape(P, KT * B)
    )

    in_maps = []
    W0 = W[0]  # [I, J, K, U]
    for c in range(NC):
        Wc = W0[:, :, c * KPC : (c + 1) * KPC, :]          # [I, J, KPC, U]
        wm = Wc.transpose(0, 3, 2, 1).reshape(KK, N)       # [(i,u), (kk,j)]
        wh = np.ascontiguousarray(
            wm.reshape(NCH, CH, P, N).transpose(0, 2, 1, 3).reshape(NCH, P, CH * N)
        )
        in_maps.append({"x": xh, "w": wh})
    return in_maps


def gather_output(results):
    """Per-core "v" [B, KPC, J] -> full [B, J, K]."""
    out = np.empty((B, J, K), dtype=np.float32)
    for c in range(NC):
        out[:, :, c * KPC : (c + 1) * KPC] = results[c]["v"].transpose(0, 2, 1)
    return out


def run(x, W, **spmd_kwargs):
    from concourse import bass_utils

    nc = get_nc()
    in_maps = prep_inputs(x, W)
    res = bass_utils.run_bass_kernel_spmd(
        nc, in_maps, core_ids=list(range(NC)), **spmd_kwargs
    )
    return gather_output(res.results), res


def kernel(x, W):
    out, _ = run(x, W)
    return out


# revision 2
# speedup vs baseline: 1.1016x; 1.1016x over previous
"""CapsuleLayer kernel for Trainium2, 8 NeuronCores.

Math: the reference's softmax is over a singleton axis, so c_ij == 1 and the
routing loop is dead code.  The output is exactly

    s[b, j, k]  = sum_{i, u} W[0, i, j, k, u] * x[b, u, i]
    m[b, k]     = sum_j s[b, j, k]^2
    v[b, j, k]  = (sqrt(m) / (1 + m)) * s[b, j, k]        (squash)

i.e. one (32 x 32768) @ (32768 x 1024) fp32 matmul plus a tiny per-(b,k)
epilogue.  W (128 MiB) dominates: the kernel is HBM-bound on reading W once.

Sharding: the output column grid is (k, j) with k = unit_size (64).  Shard on
k: core c owns k in [8c, 8c+8).  Each core reads its W slice (16 MiB, read
exactly once machine-wide), the full x (4 MiB, replicated), and computes a
fully local squash (the j-reduction inside m is intact per core).  Zero
cross-core communication.

Host-side, W is resliced into the exact SBUF streaming layout
[chunk][partition=contraction%128][ktile-in-chunk x column] so every DMA is a
single large contiguous transfer.
"""

import numpy as np

B, U, I, J, K = 32, 16, 2048, 16, 64  # batch, in_units, in_ch, num_units, unit_size
NC = 8                                # cores
KPC = K // NC                         # unit_size columns per core (8)
N = KPC * J                           # output columns per core (128), kk-major, j-minor
KK = I * U                            # contraction length (32768)
P = 128                               # partitions
KT = KK // P                          # contraction tiles (256)
CH = 16                               # contraction tiles per DMA chunk (1 MiB chunks)
NCH = KT // CH                        # chunks per core (16)

_CACHE = {}


def _build():
    import concourse.bacc as bacc
    import concourse.tile as tile
    import concourse.mybir as mybir

    f32 = mybir.dt.float32
    nc = bacc.Bacc("TRN2", num_devices=NC, debug=False, enable_asserts=False)
    x_d = nc.dram_tensor("x", (P, KT * B), f32, kind="ExternalInput")
    w_d = nc.dram_tensor("w", (NCH, P, CH * N), f32, kind="ExternalInput")
    v_d = nc.dram_tensor("v", (B, KPC, J), f32, kind="ExternalOutput")

    with tile.TileContext(nc) as tc:
        with (
            tc.tile_pool(name="xp", bufs=3) as xp,
            tc.tile_pool(name="wp", bufs=4) as wp,
            tc.tile_pool(name="ep", bufs=1) as ep,
            tc.tile_pool(name="ps", bufs=1, space="PSUM") as ps,
        ):
            s_ps = ps.tile([B, KPC, J], f32)
            for c in range(NCH):
                x_sb = xp.tile([P, CH * B], f32)
                nc.sync.dma_start(x_sb[:], x_d[:, c * CH * B : (c + 1) * CH * B])
                w_sb = wp.tile([P, CH * N], f32)
                nc.sync.dma_start(w_sb[:], w_d[c])
                for t in range(CH):
                    kt = c * CH + t
                    nc.tensor.matmul(
                        s_ps[:, :, :],
                        x_sb[:, t * B : (t + 1) * B],
                        w_sb[:, t * N : (t + 1) * N],
                        start=(kt == 0),
                        stop=(kt == KT - 1),
                    )

            s_sb = ep.tile([B, KPC, J], f32)
            nc.vector.tensor_copy(s_sb[:], s_ps[:])
            s2 = ep.tile([B, KPC, J], f32)
            nc.scalar.square(s2[:], s_ps[:])
            m = ep.tile([B, KPC], f32)
            nc.vector.reduce_sum(m[:], s2[:], axis=mybir.AxisListType.X)
            sq = ep.tile([B, KPC], f32)
            nc.scalar.sqrt(sq[:], m[:])
            d = ep.tile([B, KPC], f32)
            nc.vector.tensor_scalar_add(d[:], m[:], 1.0)
            r = ep.tile([B, KPC], f32)
            nc.vector.reciprocal(r[:], d[:])
            sc = ep.tile([B, KPC], f32)
            nc.vector.tensor_mul(sc[:], sq[:], r[:])
            v_sb = ep.tile([B, KPC, J], f32)
            for kk in range(KPC):
                nc.vector.tensor_scalar_mul(
                    v_sb[:, kk, :], s_sb[:, kk, :], sc[:, kk : kk + 1]
                )
            nc.sync.dma_start(v_d[:], v_sb[:])

    nc.compile()
    return nc


def get_nc():
    if "nc" not in _CACHE:
        _CACHE["nc"] = _build()
    return _CACHE["nc"]


def prep_inputs(x, W):
    """Full inputs -> per-core in_maps with the streaming layouts."""
    x = np.ascontiguousarray(np.asarray(x, dtype=np.float32))
    W = np.asarray(W, dtype=np.float32)
    assert x.shape == (B, U, I) and W.shape == (1, I, J, K, U)

    # x[b,u,i] -> [KK=(i major, u minor), b] -> [P, KT*B] (partition-major tiles)
    xm = x.transpose(2, 1, 0).reshape(KK, B)
    xh = np.ascontiguousarray(
        xm.reshape(KT, P, B).transpose(1, 0, 2).reshape(P, KT * B)
    )

    in_maps = []
    W0 = W[0]  # [I, J, K, U]
    for c in range(NC):
        Wc = W0[:, :, c * KPC : (c + 1) * KPC, :]          # [I, J, KPC, U]
        wm = Wc.transpose(0, 3, 2, 1).reshape(KK, N)       # [(i,u), (kk,j)]
        wh = np.ascontiguousarray(
            wm.reshape(NCH, CH, P, N).transpose(0, 2, 1, 3).reshape(NCH, P, CH * N)
        )
        in_maps.append({"x": xh, "w": wh})
    return in_maps


def gather_output(results):
    """Per-core "v" [B, KPC, J] -> full [B, J, K]."""
    out = np.empty((B, J, K), dtype=np.float32)
    for c in range(NC):
        out[:, :, c * KPC : (c + 1) * KPC] = results[c]["v"].transpose(0, 2, 1)
    return out


def run(x, W, **spmd_kwargs):
    from concourse import bass_utils

    nc = get_nc()
    in_maps = prep_inputs(x, W)
    res = bass_utils.run_bass_kernel_spmd(
        nc, in_maps, core_ids=list(range(NC)), **spmd_kwargs
    )
    return gather_output(res.results), res


def kernel(x, W):
    out, _ = run(x, W)
    return out


# revision 5
# speedup vs baseline: 1.1189x; 1.0157x over previous
"""CapsuleLayer kernel for Trainium2, 8 NeuronCores.

Math: the reference's softmax is over a singleton axis, so c_ij == 1 and the
routing loop is dead code.  The output is exactly

    s[b, j, k]  = sum_{i, u} W[0, i, j, k, u] * x[b, u, i]
    m[b, k]     = sum_j s[b, j, k]^2
    v[b, j, k]  = (sqrt(m) / (1 + m)) * s[b, j, k]        (squash)

i.e. one (32 x 32768) @ (32768 x 1024) matmul plus a tiny per-(b,k)
epilogue.  W (128 MiB) dominates: the kernel is HBM-bound on reading W once.

Sharding: the output column grid is (k, j) with k = unit_size (64).  Shard on
k: core c owns k in [8c, 8c+8).  Each core reads its W slice (16 MiB, read
exactly once machine-wide), the full x (4 MiB, replicated), and computes a
fully local squash (the j-reduction inside m is intact per core).  Zero
cross-core communication.

Numerics/PE: operands are split bf16 hi/lo pairs (x = xh + xl, W = Wh + Wl).
Each contraction tile does ONE matmul: stationary [xh|xl] (64 cols), moving
[Wh|Wl] (256 cols).  PSUM accumulates all four cross products in fp32, which
equals (xh+xl)@(Wh+Wl) exactly, i.e. fp32-grade accuracy (~1e-5) at bf16 PE
speed and with a single weight load per tile.  The epilogue folds the four
partition/column blocks together before the squash.

Host-side, W is resliced into the exact SBUF streaming layout
[chunk][partition=contraction%128][ktile-in-chunk x column] so every DMA is a
single large contiguous transfer.
"""

import numpy as np

B, U, I, J, K = 32, 16, 2048, 16, 64  # batch, in_units, in_ch, num_units, unit_size
NC = 8                                # cores
KPC = K // NC                         # unit_size columns per core (8)
N = KPC * J                           # output columns per core (128), kk-major, j-minor
KK = I * U                            # contraction length (32768)
P = 128                               # partitions
KT = KK // P                          # contraction tiles (256)
CH = 32                               # contraction tiles per DMA chunk (2 MiB chunks)
NCH = KT // CH                        # chunks per core (8)

_CACHE = {}


def _build():
    import concourse.bacc as bacc
    import concourse.tile as tile
    import concourse.mybir as mybir

    f32 = mybir.dt.float32
    bf16 = mybir.dt.bfloat16
    nc = bacc.Bacc("TRN2", num_devices=NC, debug=False, enable_asserts=False)
    # x: per k-tile [128, 64] = [xh cols 0:32 | xl cols 32:64], bf16
    x_d = nc.dram_tensor("x", (P, KT * 2 * B), bf16, kind="ExternalInput")
    # w: per k-tile [128, 256] = [Wh cols 0:128 | Wl cols 128:256], bf16
    w_d = nc.dram_tensor("w", (NCH, P, CH * 2 * N), bf16, kind="ExternalInput")
    v_d = nc.dram_tensor("v", (B, KPC, J), f32, kind="ExternalOutput")

    M = 2 * B    # stationary columns / psum partitions (64)
    NW = 2 * N   # moving columns per k-tile (256)

    with tile.TileContext(nc) as tc:
        with (
            tc.tile_pool(name="xp", bufs=3) as xp,
            tc.tile_pool(name="wp", bufs=3) as wp,
            tc.tile_pool(name="ep", bufs=1) as ep,
            tc.tile_pool(name="ps", bufs=1, space="PSUM") as ps,
        ):
            s_ps = ps.tile([M, 2, KPC, J], f32)
            for c in range(NCH):
                x_sb = xp.tile([P, CH * M], bf16)
                nc.sync.dma_start(x_sb[:], x_d[:, c * CH * M : (c + 1) * CH * M])
                w_sb = wp.tile([P, CH * NW], bf16)
                nc.sync.dma_start(w_sb[:], w_d[c])
                for t in range(CH):
                    kt = c * CH + t
                    nc.tensor.matmul(
                        s_ps[:, :, :, :],
                        x_sb[:, t * M : (t + 1) * M],
                        w_sb[:, t * NW : (t + 1) * NW],
                        start=(kt == 0),
                        stop=(kt == KT - 1),
                    )

            # fold the 4 cross products: rows (xh: 0:32, xl: 32:64),
            # cols (Wh: half 0, Wl: half 1).  DVE can't mix base partitions,
            # so shift the xl rows down to partition 0 with a tiny DMA first.
            cp = ep.tile([M, 2, KPC, J], f32)
            nc.vector.tensor_copy(cp[:], s_ps[:])
            lo = ep.tile([B, 2, KPC, J], f32)
            nc.sync.dma_start(lo[:], cp[B:M])
            t1 = ep.tile([B, 2, KPC, J], f32)
            nc.vector.tensor_add(t1[:], cp[0:B], lo[:])
            s_sb = ep.tile([B, KPC, J], f32)
            nc.vector.tensor_add(s_sb[:], t1[:, 0], t1[:, 1])

            s2 = ep.tile([B, KPC, J], f32)
            nc.scalar.square(s2[:], s_sb[:])
            m = ep.tile([B, KPC], f32)
            nc.vector.reduce_sum(m[:], s2[:], axis=mybir.AxisListType.X)
            sq = ep.tile([B, KPC], f32)
            nc.scalar.sqrt(sq[:], m[:])
            d = ep.tile([B, KPC], f32)
            nc.vector.tensor_scalar_add(d[:], m[:], 1.0)
            r = ep.tile([B, KPC], f32)
            nc.vector.reciprocal(r[:], d[:])
            sc = ep.tile([B, KPC], f32)
            nc.vector.tensor_mul(sc[:], sq[:], r[:])
            v_sb = ep.tile([B, KPC, J], f32)
            for kk in range(KPC):
                nc.vector.tensor_scalar_mul(
                    v_sb[:, kk, :], s_sb[:, kk, :], sc[:, kk : kk + 1]
                )
            nc.sync.dma_start(v_d[:], v_sb[:])

    nc.compile()
    return nc


def get_nc():
    if "nc" not in _CACHE:
        _CACHE["nc"] = _build()
    return _CACHE["nc"]


def _hi_lo(a):
    """fp32 array -> (bf16 hi, bf16 lo) with a ~= hi + lo."""
    import ml_dtypes

    hi = a.astype(ml_dtypes.bfloat16)
    lo = (a - hi.astype(np.float32)).astype(ml_dtypes.bfloat16)
    return hi, lo


def prep_inputs(x, W):
    """Full inputs -> per-core in_maps with the bf16 hi/lo streaming layouts."""
    x = np.ascontiguousarray(np.asarray(x, dtype=np.float32))
    W = np.asarray(W, dtype=np.float32)
    assert x.shape == (B, U, I) and W.shape == (1, I, J, K, U)

    # x[b,u,i] -> [KK=(i major, u minor), b] -> hi/lo pair [P, KT*2*B]
    xm = x.transpose(2, 1, 0).reshape(KT, P, B)
    xh, xl = _hi_lo(xm)
    xpair = np.stack([xh, xl], axis=2)              # [KT, P, 2, B]
    xhost = np.ascontiguousarray(
        xpair.transpose(1, 0, 2, 3).reshape(P, KT * 2 * B)
    )

    in_maps = []
    W0 = W[0]  # [I, J, K, U]
    for c in range(NC):
        Wc = W0[:, :, c * KPC : (c + 1) * KPC, :]          # [I, J, KPC, U]
        wm = Wc.transpose(0, 3, 2, 1).reshape(NCH, CH, P, N)  # [(i,u) tiled, (kk,j)]
        wh, wl = _hi_lo(wm)
        wpair = np.stack([wh, wl], axis=3)                 # [NCH, CH, P, 2, N]
        whost = np.ascontiguousarray(
            wpair.transpose(0, 2, 1, 3, 4).reshape(NCH, P, CH * 2 * N)
        )
        in_maps.append({"x": xhost, "w": whost})
    return in_maps


def gather_output(results):
    """Per-core "v" [B, KPC, J] -> full [B, J, K]."""
    out = np.empty((B, J, K), dtype=np.float32)
    for c in range(NC):
        out[:, :, c * KPC : (c + 1) * KPC] = results[c]["v"].transpose(0, 2, 1)
    return out


def run(x, W, **spmd_kwargs):
    from concourse import bass_utils

    nc = get_nc()
    in_maps = prep_inputs(x, W)
    res = bass_utils.run_bass_kernel_spmd(
        nc, in_maps, core_ids=list(range(NC)), **spmd_kwargs
    )
    return gather_output(res.results), res


def kernel(x, W):
    out, _ = run(x, W)
    return out


# revision 9
# speedup vs baseline: 1.2411x; 1.1092x over previous
"""CapsuleLayer kernel for Trainium2, 8 NeuronCores.

Math: the reference's softmax is over a singleton axis, so c_ij == 1 and the
routing loop is dead code.  The output is exactly

    s[b, j, k]  = sum_{i, u} W[0, i, j, k, u] * x[b, u, i]
    m[b, k]     = sum_j s[b, j, k]^2
    v[b, j, k]  = (sqrt(m) / (1 + m)) * s[b, j, k]        (squash)

i.e. one (32 x 32768) @ (32768 x 1024) matmul plus a tiny per-(b,k)
epilogue.  W (128 MiB) dominates: the kernel is HBM-bound on reading W once.

Sharding: the output column grid is (k, j) with k = unit_size (64).  Shard on
k: core c owns k in [8c, 8c+8).  Each core reads its W slice (16 MiB, read
exactly once machine-wide), the full x (4 MiB, replicated), and computes a
fully local squash (the j-reduction inside m is intact per core).  Zero
cross-core communication.

Numerics/PE: operands are split bf16 hi/lo pairs (x = xh + xl, W = Wh + Wl).
Each contraction tile does ONE matmul: stationary [xh|xl] (64 cols), moving
[Wh|Wl] (256 cols).  PSUM accumulates all four cross products in fp32, which
equals (xh+xl)@(Wh+Wl) exactly, i.e. fp32-grade accuracy (~1e-5) at bf16 PE
speed and with a single weight load per tile.  The epilogue folds the four
partition/column blocks together before the squash.

Host-side, W is resliced into the exact SBUF streaming layout
[chunk][partition=contraction%128][ktile-in-chunk x column] so every DMA is a
single large contiguous transfer.
"""

import numpy as np

B, U, I, J, K = 32, 16, 2048, 16, 64  # batch, in_units, in_ch, num_units, unit_size
NC = 8                                # cores
KPC = K // NC                         # unit_size columns per core (8)
N = KPC * J                           # output columns per core (128), kk-major, j-minor
KK = I * U                            # contraction length (32768)
P = 128                               # partitions
KT = KK // P                          # contraction tiles (256)
# Chunk sizes (in contraction tiles): small first chunks so the PE starts
# ~4us into the kernel instead of waiting on a full 2 MiB transfer.
CHUNKS = [2, 2, 4, 8, 16] + [32] * 7
assert sum(CHUNKS) == KT

_CACHE = {}


def _build():
    import concourse.bacc as bacc
    import concourse.tile as tile
    import concourse.mybir as mybir

    import concourse.bass as bass

    f32 = mybir.dt.float32
    bf16 = mybir.dt.bfloat16
    nc = bacc.Bacc("TRN2", num_devices=NC, debug=False, enable_asserts=False)
    # x: per k-tile [128, 64] = [xh cols 0:32 | xl cols 32:64], bf16
    x_d = nc.dram_tensor("x", (P, KT * 2 * B), bf16, kind="ExternalInput")
    # w: per k-tile [128, 256] = [Wh cols 0:128 | Wl cols 128:256], bf16
    w_d = nc.dram_tensor("w", (P, KT * 2 * N), bf16, kind="ExternalInput")
    v_d = nc.dram_tensor("v", (B, KPC, J), f32, kind="ExternalOutput")

    M = 2 * B    # stationary columns / psum partitions (64)
    NW = 2 * N   # moving columns per k-tile (256)

    with tile.TileContext(nc) as tc:
        with (
            tc.tile_pool(name="xp", bufs=4) as xp,
            tc.tile_pool(name="wp", bufs=4) as wp,
            tc.tile_pool(name="ep", bufs=1) as ep,
            tc.tile_pool(name="ps", bufs=1, space="PSUM") as ps,
        ):
            s_ps = ps.tile([M, 2, KPC, J], f32)
            kt0 = 0
            for ch in CHUNKS:
                x_sb = xp.tile([P, 32 * M], bf16, tag="xch")
                nc.sync.dma_start(
                    x_sb[:, : ch * M],
                    x_d[:, kt0 * M : (kt0 + ch) * M],
                )
                w_sb = wp.tile([P, 32 * NW], bf16, tag="wch")
                nc.sync.dma_start(
                    w_sb[:, : ch * NW],
                    w_d[:, kt0 * NW : (kt0 + ch) * NW],
                )
                for t in range(ch):
                    kt = kt0 + t
                    nc.tensor.matmul(
                        s_ps[:, :, :, :],
                        x_sb[:, t * M : (t + 1) * M],
                        w_sb[:, t * NW : (t + 1) * NW],
                        start=(kt == 0),
                        stop=(kt == KT - 1),
                    )
                kt0 += ch

            # fold the 4 cross products: rows (xh: 0:32, xl: 32:64),
            # cols (Wh: half 0, Wl: half 1).  DVE can't mix base partitions,
            # so shift the xl rows down to partition 0 with a tiny DMA first.
            cp = ep.tile([M, 2, KPC, J], f32)
            nc.vector.tensor_copy(cp[:], s_ps[:])
            lo = ep.tile([B, 2, KPC, J], f32)
            nc.sync.dma_start(lo[:], cp[B:M])
            t1 = ep.tile([B, 2, KPC, J], f32)
            nc.vector.tensor_add(t1[:], cp[0:B], lo[:])
            s_sb = ep.tile([B, KPC, J], f32)
            nc.vector.tensor_add(s_sb[:], t1[:, 0], t1[:, 1])

            s2 = ep.tile([B, KPC, J], f32)
            nc.scalar.square(s2[:], s_sb[:])
            m = ep.tile([B, KPC], f32)
            nc.vector.reduce_sum(m[:], s2[:], axis=mybir.AxisListType.X)
            sq = ep.tile([B, KPC], f32)
            nc.scalar.sqrt(sq[:], m[:])
            d = ep.tile([B, KPC], f32)
            nc.vector.tensor_scalar_add(d[:], m[:], 1.0)
            r = ep.tile([B, KPC], f32)
            nc.vector.reciprocal(r[:], d[:])
            sc = ep.tile([B, KPC], f32)
            nc.vector.tensor_mul(sc[:], sq[:], r[:])
            v_sb = ep.tile([B, KPC, J], f32)
            sc_ap = sc[:]
            sc_bc = bass.AP(
                sc_ap.tensor,
                sc_ap.offset,
                [list(sc_ap.ap[0]), list(sc_ap.ap[1]), [0, J]],
            )
            nc.vector.tensor_mul(v_sb[:], s_sb[:], sc_bc)
            nc.sync.dma_start(v_d[:], v_sb[:])

    nc.compile()
    return nc


def get_nc():
    if "nc" not in _CACHE:
        _CACHE["nc"] = _build()
    return _CACHE["nc"]


def _hi_lo(a):
    """fp32 array -> (bf16 hi, bf16 lo) with a ~= hi + lo."""
    import ml_dtypes

    hi = a.astype(ml_dtypes.bfloat16)
    lo = (a - hi.astype(np.float32)).astype(ml_dtypes.bfloat16)
    return hi, lo


def prep_inputs(x, W):
    """Full inputs -> per-core in_maps with the bf16 hi/lo streaming layouts."""
    x = np.ascontiguousarray(np.asarray(x, dtype=np.float32))
    W = np.asarray(W, dtype=np.float32)
    assert x.shape == (B, U, I) and W.shape == (1, I, J, K, U)

    # x[b,u,i] -> [KK=(i major, u minor), b] -> hi/lo pair [P, KT*2*B]
    xm = x.transpose(2, 1, 0).reshape(KT, P, B)
    xh, xl = _hi_lo(xm)
    xpair = np.stack([xh, xl], axis=2)              # [KT, P, 2, B]
    xhost = np.ascontiguousarray(
        xpair.transpose(1, 0, 2, 3).reshape(P, KT * 2 * B)
    )

    in_maps = []
    W0 = W[0]  # [I, J, K, U]
    for c in range(NC):
        Wc = W0[:, :, c * KPC : (c + 1) * KPC, :]          # [I, J, KPC, U]
        wm = Wc.transpose(0, 3, 2, 1).reshape(KT, P, N)    # [(i,u) tiled, (kk,j)]
        wh, wl = _hi_lo(wm)
        wpair = np.stack([wh, wl], axis=2)                 # [KT, P, 2, N]
        whost = np.ascontiguousarray(
            wpair.transpose(1, 0, 2, 3).reshape(P, KT * 2 * N)
        )
        in_maps.append({"x": xhost, "w": whost})
    return in_maps


def gather_output(results):
    """Per-core "v" [B, KPC, J] -> full [B, J, K]."""
    out = np.empty((B, J, K), dtype=np.float32)
    for c in range(NC):
        out[:, :, c * KPC : (c + 1) * KPC] = results[c]["v"].transpose(0, 2, 1)
    return out


def run(x, W, **spmd_kwargs):
    from concourse import bass_utils

    nc = get_nc()
    in_maps = prep_inputs(x, W)
    res = bass_utils.run_bass_kernel_spmd(
        nc, in_maps, core_ids=list(range(NC)), **spmd_kwargs
    )
    return gather_output(res.results), res


def kernel(x, W):
    out, _ = run(x, W)
    return out
